# revision 12
# baseline (speedup 1.0000x reference)
"""Distributed Trainium2 Bass kernel for AlignmentContrastiveLoss (v3).

Reference computation (B=256, L_im=37, L_s=33, D=1024):
    im  = l2norm(im_set)[:, 1:, :]   masked by im_len-1     [B, 36, D]
    s   = l2norm(s_seq)[:, 1:-2, :]  masked by s_len-3      [B, 30, D]
    align[b,c,i,j] = im[b,i] . s[c,j]   (masked entries -> 0)
    scores[b,c] = sum_j max_i align[b,c,i,j]
    loss = sum_b relu(M + max_{c!=b} scores[b,c] - scores[b,b])
         + sum_c relu(M + max_{b!=c} scores[b,c] - scores[c,c])

v3 strategy (vs v2's 117us):
  * All prep moves to the host: im AND s rows are l2-normalized, scaled
    x16 and cast to fp8 e4m3 in numpy; im ships pre-transposed in the
    exact SBUF layout.  The device runs ONLY the fp8 DoubleRow align
    matmuls, the DVE max-reduce, the tiny G accumulation and the hinge
    stats.  (v2 spent the first 16us of the kernel on device-side im
    normalization before the PE could start, plus per-tile gram matmuls
    + diag extraction + sqrt/reciprocal for the s norms.)
  * No s-scale anywhere: with s normalized on the host the G matrix
    entries are exactly 1/256 (power of two, exact in bf16), which
    cancels the 16*16 fp8 scaling, so s_acc accumulates scores at scale
    1 and the v2 hinge-stats epilogue is reused verbatim.
  * s rows are compacted globally (not per 128-sentence half): NT drops
    36 -> 35; the single half-boundary tile issues two G matmuls.
  * PSUM packing is flat: one [128, 512*NBANK] accumulation tile, im
    rows packed contiguously; matmuls split at bank boundaries (512
    f32), the DVE reduces view the flat range and may span banks, so a
    tile needs exactly one reduce instruction per R-class (4 of them).
  * mx is written by the DVE directly as bf16, feeding the G matmul
    with no scalar-engine hop.
  * DMA ramp: imt/gmat are split into 8-partition chunks issued from
    the scalar/gpsimd/vector queues in parallel with the sync queue's
    st stream (the first tiles split 4-way) so the first align matmul
    can start as soon as possible.
"""

import os
import sys

import numpy as np
import ml_dtypes

for _p in ("/opt/trn_rl_repo", "/root/.axon_site/_ro/trn_rl_repo"):
    if os.path.isdir(_p) and _p not in sys.path:
        sys.path.append(_p)

import concourse.bass as bass
import concourse.mybir as mybir
import concourse.tile as tile
from concourse import bacc
from concourse.bass_utils import run_bass_kernel_spmd


def _ensure_axon_hooks():
    """Some agent images ship an ``antenv`` without ``axon_hooks``, but
    bass_utils hard-imports it when trace=True.  Provide the registry and,
    when libaxon_pjrt.so is available, the real NTFF profile hook."""
    import types

    try:
        import antenv.axon_hooks  # noqa: F401
        return
    except ImportError:
        pass
    try:
        import antenv
    except ImportError:
        return
    mod = types.ModuleType("antenv.axon_hooks")
    mod._hook = None
    mod.set_axon_ntff_profile_hook = lambda h: setattr(mod, "_hook", h)
    mod.get_axon_ntff_profile_hook = lambda: mod._hook
    sys.modules["antenv.axon_hooks"] = mod
    antenv.axon_hooks = mod
    so_path = "/opt/axon/libaxon_pjrt.so"
    try:
        import trn_agent_boot.trn_boot as _tb
        if os.path.exists(so_path):
            mod._hook = _tb._ntff_profile_via_ctypes(so_path)
    except Exception:
        pass


_ensure_axon_hooks()

F32 = mybir.dt.float32
F32R = mybir.dt.float32r
BF16 = mybir.dt.bfloat16
F8 = mybir.dt.float8e4
I32 = mybir.dt.int32
AX = mybir.AxisListType
ALU = mybir.AluOpType
ACT = mybir.ActivationFunctionType
DR = mybir.MatmulPerfMode.DoubleRow

NCORES = 8
B, LI, LS, D = 256, 36, 30, 1024
KC = D // 128               # 8 contraction chunks of 128
G = 6                       # im row-padding granularity
MARGIN, EPS, NEG = 0.2, 1e-12, -1.0e9
GLAG = 8                    # tiles of lag before a tile's G matmul
SLAG = 3                    # extra lag for the stats PE-transpose part
GSC = 1.0 / 256.0           # exact in bf16; cancels the 16*16 fp8 scale
N_JUNK = int(os.environ.get("N_JUNK", "8"))  # PE warm-up matmuls

LAST_RESULT = None  # BassKernelResults of the most recent run (for test harness)

# Dedup redundant PE weight loads: bass lowering splits every matmul into a
# standalone Ldweights + non-self-loading Matmult, but emits one Ldweights
# per matmul even when consecutive matmuls share the same stationary
# operand.  We post-process the BIR json and drop a generated Ldweights
# (no semaphore waits/updates) when the weights signature matches what the
# PE already has loaded.
LDW_DEDUP = os.environ.get("LDW_DEDUP", "1") == "1"


def _dedup_ldweights_json(js_bytes):
    import json as _json

    j = _json.loads(js_bytes)
    dropped = 0
    for fn in j.get("functions", []):
        for blk in fn.get("blocks", []):
            insts = blk.get("instructions")
            if not insts:
                continue
            out = []
            loaded = None
            for x in insts:
                if x.get("engine") != "PE":
                    out.append(x)
                    continue
                op = x.get("opcode")
                if op == "Ldweights":
                    sig = _json.dumps(
                        [x.get("ins"), x.get("perf_mode"),
                         x.get("tile_size"), x.get("tile_position"),
                         x.get("is_transpose")], sort_keys=True)
                    sync = x.get("sync_info") or {}
                    if (sig == loaded and not sync.get("on_wait")
                            and not sync.get("on_update")):
                        dropped += 1
                        continue
                    loaded = sig
                    out.append(x)
                elif op == "Matmult":
                    if x.get("ldweights") is not False:
                        loaded = None  # self-loading matmul clobbers weights
                    out.append(x)
                else:
                    loaded = None
                    out.append(x)
            blk["instructions"] = out
    return _json.dumps(j).encode(), dropped


# ---------------------------------------------------------------------------
# layout planning (data-dependent, host side)
# ---------------------------------------------------------------------------

class Plan:
    pass


def plan_layout(im_l, s_l):
    p = Plan()
    # ---- s side: globally compacted row list ----
    rows = [(c, j) for c in range(B) for j in range(int(s_l[c]))]
    NT = -(-len(rows) // 128)
    rows = rows + [None] * (NT * 128 - len(rows))
    p.NT = NT
    p.srows = rows
    # per-tile sentence-half blocks for the G matmul.  Block t of gmat is
    # tile t's primary half; the (at most one) tile straddling the
    # c=127/128 boundary gets a second block at index NT.
    p.g_emits = []
    for t in range(NT):
        tr = [r for r in rows[128 * t:128 * t + 128] if r is not None]
        halves = sorted({r[0] // 128 for r in tr})
        ge = [(halves[0], t)]
        if len(halves) == 2:
            ge.append((halves[1], NT))
        p.g_emits.append(ge)
    p.first_t, p.last_t = {}, {}
    for t, ge in enumerate(p.g_emits):
        for h, _ in ge:
            p.first_t.setdefault(h, t)
            p.last_t[h] = t

    # ---- im side: R template shared across cores ----
    # R >= im_l+1 (>=1 zero row emulates the reference's max-includes-zero
    # mask) unless im_l == LI; multiple of G, clamped >= 18 so the template
    # has at most 4 R-classes -> 4 DVE reduce instructions per tile.
    R = np.where(im_l >= LI, LI,
                 (G * np.ceil((im_l + 1) / G)).astype(np.int64)).astype(np.int64)
    R = np.maximum(R, min(18, LI))
    order = np.argsort(-R, kind="stable")
    p.order = order                       # slot i of core m -> image order[8i+m]
    p.template = [int(R[order[8 * i]]) for i in range(32)]
    off = np.concatenate([[0], np.cumsum(p.template)]).astype(int)
    p.slot_off = off
    p.NR = int(off[32])
    p.NBANK = -(-p.NR // 512)
    assert p.NBANK * 512 <= 2048
    # reduce segments: runs of equal R (descending template -> contiguous)
    segs = []
    i = 0
    while i < 32:
        j = i
        while j < 32 and p.template[j] == p.template[i]:
            j += 1
        segs.append({"off": int(off[i]), "n": j - i, "R": p.template[i],
                     "mxoff": i})
        i = j
    p.segs = segs
    return p


def _plan_key(p):
    return (p.NT, p.NR, p.NBANK, tuple(p.template),
            tuple((t, h, blk) for t, ge in enumerate(p.g_emits)
                  for h, blk in ge))


# ---------------------------------------------------------------------------
# device program
# ---------------------------------------------------------------------------

def build_nc(p):
    NT, NR, NBANK = p.NT, p.NR, p.NBANK

    nc = bacc.Bacc(None, target_bir_lowering=False, debug=False,
                   num_devices=NCORES)

    imt_e = nc.declare_dram_parameter("imt", [128, KC * NR], F8,
                                      isOutput=False)
    st_e = nc.declare_dram_parameter("st", [NT, 128, KC, 128], F8,
                                     isOutput=False)
    gmat_e = nc.declare_dram_parameter("gmat", [128, (NT + 1) * 128], BF16,
                                       isOutput=False)
    ident_e = nc.declare_dram_parameter("ident", [128, 128], F32,
                                        isOutput=False)
    pos0_e = nc.declare_dram_parameter("pos0", [128, 32], F32, isOutput=False)
    pos1_e = nc.declare_dram_parameter("pos1", [128, 32], F32, isOutput=False)
    post0_e = nc.declare_dram_parameter("post0", [32, 128], F32, isOutput=False)
    post1_e = nc.declare_dram_parameter("post1", [32, 128], F32, isOutput=False)
    out_e = nc.declare_dram_parameter("out", [128, 6], F32, isOutput=True)

    with tile.TileContext(nc) as tc:
        from contextlib import ExitStack

        with ExitStack() as ctx:
            const = ctx.enter_context(tc.tile_pool(name="const", bufs=1))
            small = ctx.enter_context(tc.tile_pool(name="small", bufs=1))
            stp = ctx.enter_context(tc.tile_pool(name="stp", bufs=8))
            mxp = ctx.enter_context(tc.tile_pool(name="mxp", bufs=GLAG + 3))
            pal = ctx.enter_context(
                tc.tile_pool(name="pal", bufs=(3 if NBANK <= 2 else 2),
                             space="PSUM"))
            pmisc = ctx.enter_context(
                tc.tile_pool(name="pmisc", bufs=1, space="PSUM"))
            psacc = ctx.enter_context(
                tc.tile_pool(name="psacc", bufs=1, space="PSUM"))

            def misc_psum(shape, name):
                return pmisc.tile(shape, F32, tag="misc", bufs=1, name=name)

            # ---- PE warm-up: junk matmuls keep the PE p-state at max and
            # absorb the DMA ramp (weights memset by gpsimd at t~0) ----
            junkw = const.tile([128, 512], BF16, tag="junkw")
            nc.gpsimd.memset(junkw[:, :], 1.0)
            if N_JUNK:
                junk_ps = pmisc.tile([128, 512], F32, tag="misc", bufs=1,
                                     name="junk_ps")
                for _ in range(N_JUNK):
                    nc.tensor.matmul(junk_ps[:, :], lhsT=junkw[:, 0:128],
                                     rhs=junkw[:, :], start=True, stop=True,
                                     skip_group_check=True)

            # ---- ramp DMAs.  A dma_start costs ~0.7us of descriptor-gen on
            # the ISSUING sequencer, so chunk counts are kept low and spread:
            # imt is 4 kp-piece tiles x 3 partition-chunks so tile 0's first
            # matmuls can start as soon as the kp0 piece lands. ----
            imt_p = [const.tile([128, 2 * NR], F8, tag=f"imt{kp}",
                                name=f"imt{kp}")
                     for kp in range(KC // 2)]
            imt3_p = [x.rearrange("p (k n) -> p k n", k=2) for x in imt_p]
            gmat = const.tile([128, (NT + 1) * 128], BF16, tag="gmat")

            piece_issuers = {0: [nc.sync, nc.scalar, nc.scalar],
                             1: [nc.sync, nc.scalar, nc.scalar],
                             2: [nc.gpsimd, nc.gpsimd, nc.gpsimd],
                             3: [nc.gpsimd, nc.gpsimd, nc.gpsimd]}
            psplit = [0, 43, 86, 128]

            def issue_imt_piece(kp, ci):
                a, b = psplit[ci], psplit[ci + 1]
                piece_issuers[kp][ci].dma_start(
                    out=imt_p[kp][a:b, :],
                    in_=imt_e[a:b, 2 * kp * NR:(2 * kp + 2) * NR])

            # ---- epilogue consts (gpsimd queue, after imt/gmat) ----
            ident = const.tile([128, 128], F32, tag="ident")
            pos0 = const.tile([128, 32], F32, tag="pos0")
            pos1 = const.tile([128, 32], F32, tag="pos1")
            post0 = const.tile([32, 128], F32, tag="post0")
            post1 = const.tile([32, 128], F32, tag="post1")
            margin128 = const.tile([128, 1], F32, tag="margin128")

            def issue_late_consts():
                # gpsimd: kp2/kp3 pieces, then gmat, then stats consts
                for kp in (2, 3):
                    for ci in range(3):
                        issue_imt_piece(kp, ci)
                for ci in range(8):
                    nc.gpsimd.dma_start(
                        out=gmat[16 * ci:16 * ci + 16, :],
                        in_=gmat_e[16 * ci:16 * ci + 16, :])
                nc.gpsimd.dma_start(out=ident[:, :], in_=ident_e[:, :])
                nc.gpsimd.dma_start(out=pos0[:, :], in_=pos0_e[:, :])
                nc.gpsimd.dma_start(out=pos1[:, :], in_=pos1_e[:, :])
                nc.gpsimd.dma_start(out=post0[:, :], in_=post0_e[:, :])
                nc.gpsimd.dma_start(out=post1[:, :], in_=post1_e[:, :])
                nc.gpsimd.memset(margin128[:, :], MARGIN)

            # scalar: its share of the kp0/kp1 pieces (nothing else early)
            for kp in (0, 1):
                for ci in (1, 2):
                    issue_imt_piece(kp, ci)
            issue_late_consts()

            posm = [pos0, pos1]
            payload = small.tile([128, 6], F32, tag="payload")
            snd = [small.tile([128, 32], F32, tag=f"snd{h}", name=f"snd{h}")
                   for h in range(2)]
            trash32 = small.tile([128, 32], F32, tag="trash32")
            negm = [small.tile([128, 32], F32, tag=f"negm{h}", name=f"negm{h}")
                    for h in range(2)]
            stats_ready = []

            def ensure_stats_consts():
                # lazy: keeps the vector queue free during the DMA ramp
                if stats_ready:
                    return
                nc.vector.tensor_scalar_mul(negm[0][:, :], pos0[:, :], NEG)
                nc.vector.tensor_scalar_mul(negm[1][:, :], pos1[:, :], NEG)
                stats_ready.append(True)

            rm = small.tile([32, 2], F32, tag="rm")

            # S accumulators: both halves share one PSUM bank
            s_acc = psacc.tile([128, 64], F32, tag="S", name="S")
            s_ps = [s_acc[:, 0:32], s_acc[:, 32:64]]

            mx_tiles = {}

            def issue_st(t):
                st_t = stp.tile([128, KC * 128], F8, tag="st")
                st3 = st_t.rearrange("p (k c) -> p k c", k=KC)
                nw = 2 if t < 6 else 1
                pp = 128 // nw
                for w in range(nw):
                    nc.sync.dma_start(
                        out=st3[pp * w:pp * (w + 1), :, :],
                        in_=st_e[t, pp * w:pp * (w + 1), :, :])
                return st_t

            def emit_tile(t, st_t):
                st3 = st_t.rearrange("p (k c) -> p k c", k=KC)
                # flat [128, NBANK*512] accumulation tile; matmuls split at
                # bank boundaries, reduces view the flat col range freely
                ps_t = pal.tile([128, NBANK * 512], F32, tag="al", name="ps")
                for kp in range(KC // 2):
                    w = st3[:, 2 * kp:2 * kp + 2, :]
                    for bi in range(NBANK):
                        c0, c1 = 512 * bi, min(512 * (bi + 1), NR)
                        nc.tensor.matmul(
                            ps_t[:, c0:c1],
                            lhsT=w,
                            rhs=imt3_p[kp][:, :, c0:c1],
                            start=(kp == 0), stop=(kp == KC // 2 - 1),
                            perf_mode=DR, skip_group_check=True,
                        )
                # max over image rows -> mx [128, 32] bf16 (feeds G matmul)
                mx = mxp.tile([128, 32], BF16, tag="mx", name="mx")
                for s in p.segs:
                    w = s["n"] * s["R"]
                    nc.vector.tensor_reduce(
                        out=mx[:, s["mxoff"]:s["mxoff"] + s["n"]],
                        in_=ps_t[:, s["off"]:s["off"] + w].rearrange(
                            "p (n r) -> p n r", r=s["R"]),
                        axis=AX.X, op=ALU.max,
                    )
                mx_tiles[t] = mx

            def emit_g(t):
                for h, blk in p.g_emits[t]:
                    nc.tensor.matmul(
                        s_ps[h],
                        lhsT=gmat[:, 128 * blk:128 * (blk + 1)],
                        rhs=mx_tiles[t][:, :],
                        start=(t == p.first_t[h]), stop=(t == p.last_t[h]),
                    )

            def emit_stats_a(h):
                # DVE part: diag extract, diag-masked copy, local col-max
                ensure_stats_consts()
                nc.vector.scalar_tensor_tensor(
                    out=trash32[:, :], in0=s_ps[h], scalar=1.0,
                    in1=posm[h][:, :], op0=ALU.mult, op1=ALU.mult,
                    accum_out=payload[:, 2 + h:3 + h],
                )
                nc.vector.tensor_add(snd[h][:, :], s_ps[h], negm[h][:, :])
                nc.vector.tensor_reduce(out=payload[:, h:h + 1],
                                        in_=snd[h][:, :], axis=AX.X,
                                        op=ALU.max)

            def emit_stats_b(h):
                # PE transpose trails the DVE part by SLAG tiles so the
                # in-order PE never waits on the DVE's reduce backlog
                stp_ps = misc_psum([32, 128], "stp_ps")
                nc.tensor.transpose(stp_ps[:, :], snd[h][:, :], ident[:, :])
                nc.vector.tensor_reduce(out=rm[:, h:h + 1], in_=stp_ps[:, :],
                                        axis=AX.X, op=ALU.max)

            # ---- main loop (G drained with GLAG-tile lag) ----
            done_a, done_b = set(), set()

            def after_g(t):
                for h in (0, 1):
                    if t == p.last_t[h] and h not in done_a:
                        emit_stats_a(h)
                        done_a.add(h)

            def after_b(t):
                for h in (0, 1):
                    if t == p.last_t[h] and h in done_a and h not in done_b:
                        emit_stats_b(h)
                        done_b.add(h)

            # sync queue order: st0, imt-kp0 chunk, st1, imt-kp1 chunk, st2..
            # (all writers emitted before their readers in the loop below)
            st_pre = {0: issue_st(0)}
            issue_imt_piece(0, 0)
            st_pre[1] = issue_st(1)
            issue_imt_piece(1, 0)

            for t in range(NT):
                emit_tile(t, st_pre.pop(t) if t in st_pre else issue_st(t))
                if t - GLAG >= 0:
                    emit_g(t - GLAG)
                    after_g(t - GLAG)
                if t - GLAG - SLAG >= 0:
                    after_b(t - GLAG - SLAG)
            for t in range(max(0, NT - GLAG), NT):
                emit_g(t)
                after_g(t)
            for h in (0, 1):
                if h not in done_b:
                    emit_stats_b(h)
                    done_b.add(h)

            # ---- row-hinge epilogue ----
            posr = [small.tile([128, 32], F32R, tag=f"posr{h}", name=f"posr{h}")
                    for h in range(2)]
            nc.scalar.copy(posr[0][:, :], pos0[:, :])
            nc.scalar.copy(posr[1][:, :], pos1[:, :])
            postr = [small.tile([32, 128], F32R, tag=f"postr{h}",
                                name=f"postr{h}") for h in range(2)]
            nc.scalar.copy(postr[0][:, :], post0[:, :])
            nc.scalar.copy(postr[1][:, :], post1[:, :])
            rowmax = small.tile([32, 1], F32, tag="rowmax")
            nc.vector.tensor_max(rowmax[:, :], rm[:, 0:1], rm[:, 1:2])
            # own-diag per image (row order): for each half h, pos_h^T @ d_h
            dca = small.tile([128, 2], F32R, tag="dca")
            dcb = small.tile([128, 2], F32R, tag="dcb")
            nc.scalar.copy(dca[:, 0:1], payload[:, 2:3])
            nc.scalar.mul(dca[:, 1:2], payload[:, 2:3], mul=0.0)
            nc.scalar.copy(dcb[:, 0:1], payload[:, 3:4])
            nc.scalar.mul(dcb[:, 1:2], payload[:, 3:4], mul=0.0)
            dfree_ps = misc_psum([32, 2], "dfree_ps")
            nc.tensor.matmul(dfree_ps[:, :], lhsT=posr[0][:, :],
                             rhs=dca[:, :], start=True, stop=False)
            nc.tensor.matmul(dfree_ps[:, :], lhsT=posr[1][:, :],
                             rhs=dcb[:, :], start=False, stop=True)
            dfree_sb = small.tile([32, 1], F32, tag="dfree_sb")
            nc.scalar.copy(dfree_sb[:, :], dfree_ps[:, 0:1])
            rh_pre = small.tile([32, 2], F32, tag="rh_pre")
            nc.gpsimd.memset(rh_pre[:, :], 0.0)
            nc.vector.tensor_sub(rh_pre[:, 0:1], rowmax[:, :], dfree_sb[:, :])
            rowhinge = small.tile([32, 2], F32R, tag="rowhinge")
            nc.scalar.activation(rowhinge[:, :], rh_pre[:, :], ACT.Relu,
                                 bias=margin128[0:32, :])
            for h in range(2):
                rh_ps = misc_psum([128, 2], "rh_ps")
                nc.tensor.matmul(rh_ps[:, :], lhsT=postr[h][:, :],
                                 rhs=rowhinge[:, :], start=True, stop=True)
                nc.scalar.copy(payload[:, 4 + h:5 + h], rh_ps[:, 0:1])

            nc.sync.dma_start(out=out_e[:, :], in_=payload[:, :])

    nc.finalize()
    return nc


# ---------------------------------------------------------------------------
# host side
# ---------------------------------------------------------------------------

def build_in_maps(p, im_set, s_seq):
    im_set = np.asarray(im_set, dtype=np.float32)
    s_seq = np.asarray(s_seq, dtype=np.float32)
    NT, NR = p.NT, p.NR

    # s tiles (shared): fp8 of 16*l2norm(word rows) in compacted order
    sn = s_seq / np.maximum(
        np.linalg.norm(s_seq, axis=2, keepdims=True), EPS)
    srows = np.zeros((NT * 128, D), dtype=np.float32)
    gmat = np.zeros((128, (NT + 1) * 128), dtype=np.float32)
    for i, cj in enumerate(p.srows):
        if cj is None:
            continue
        c, j = cj
        srows[i] = 16.0 * sn[c, 1 + j]
        t, pp = divmod(i, 128)
        h = c // 128
        blk = None
        for hh, bb in p.g_emits[t]:
            if hh == h:
                blk = bb
        gmat[pp, 128 * blk + (c % 128)] = GSC
    s8 = srows.astype(ml_dtypes.float8_e4m3)
    st = np.ascontiguousarray(
        s8.reshape(NT, 128, KC, 128).transpose(0, 3, 2, 1))
    gmat = gmat.astype(ml_dtypes.bfloat16)

    ident = np.eye(128, dtype=np.float32)

    imn = im_set / np.maximum(
        np.linalg.norm(im_set, axis=2, keepdims=True), EPS)

    in_maps = []
    for m in range(NCORES):
        imtf = np.zeros((NR, D), dtype=np.float32)
        pos0 = np.zeros((128, 32), np.float32)
        pos1 = np.zeros((128, 32), np.float32)
        for i in range(32):
            b = int(p.order[8 * i + m])
            off = int(p.slot_off[i])
            nvalid = int(p.im_l[b])
            imtf[off:off + nvalid] = 16.0 * imn[b, 1:1 + nvalid]
            if b < 128:
                pos0[b % 128, i] = 1.0
            else:
                pos1[b % 128, i] = 1.0
        imt8 = imtf.astype(ml_dtypes.float8_e4m3)
        imt = np.ascontiguousarray(
            imt8.reshape(NR, KC, 128).transpose(2, 1, 0)).reshape(128, KC * NR)
        in_maps.append({
            "imt": imt,
            "st": st,
            "gmat": gmat,
            "ident": ident,
            "pos0": pos0,
            "pos1": pos1,
            "post0": np.ascontiguousarray(pos0.T),
            "post1": np.ascontiguousarray(pos1.T),
        })
    return in_maps


def host_combine(outs):
    """Combine the 8 cores' [128, 6] payloads into the scalar loss."""
    agg = np.stack([np.asarray(o, dtype=np.float32) for o in outs])  # [8,128,6]
    colmax = agg[:, :, 0:2].max(axis=0)          # [128, 2]
    diag = agg[:, :, 2:4].sum(axis=0)            # [128, 2]
    colhinge = np.maximum(MARGIN + colmax - diag, 0.0).sum()
    rowhinge = agg[:, :, 4:6].sum()
    return np.float32(colhinge + rowhinge)


_NC_CACHE = {}


def kernel(im_set, s_seq, im_len, s_len):
    global LAST_RESULT
    im_len = np.asarray(im_len, dtype=np.int32)
    s_len = np.asarray(s_len, dtype=np.int32)
    im_l = im_len - 1
    s_l = s_len - 3

    p = plan_layout(im_l, s_l)
    p.im_l = im_l
    key = _plan_key(p)
    if key not in _NC_CACHE:
        nc = build_nc(p)
        if LDW_DEDUP:
            _orig = nc.to_json_bytes

            def _to_json_bytes_dedup(_orig=_orig):
                js, _ = _dedup_ldweights_json(_orig())
                return js

            nc.to_json_bytes = _to_json_bytes_dedup
        _NC_CACHE[key] = nc
    nc = _NC_CACHE[key]

    in_maps = build_in_maps(p, im_set, s_seq)
    res = run_bass_kernel_spmd(nc, in_maps, core_ids=list(range(NCORES)))
    LAST_RESULT = res
    return host_combine([r["out"] for r in res.results])


# revision 16
# speedup vs baseline: 1.2874x; 1.2874x over previous
"""Distributed Trainium2 Bass kernel for AlignmentContrastiveLoss (v3).

Reference computation (B=256, L_im=37, L_s=33, D=1024):
    im  = l2norm(im_set)[:, 1:, :]   masked by im_len-1     [B, 36, D]
    s   = l2norm(s_seq)[:, 1:-2, :]  masked by s_len-3      [B, 30, D]
    align[b,c,i,j] = im[b,i] . s[c,j]   (masked entries -> 0)
    scores[b,c] = sum_j max_i align[b,c,i,j]
    loss = sum_b relu(M + max_{c!=b} scores[b,c] - scores[b,b])
         + sum_c relu(M + max_{b!=c} scores[b,c] - scores[c,c])

v3 strategy (vs v2's 117us):
  * All prep moves to the host: im AND s rows are l2-normalized, scaled
    x16 and cast to fp8 e4m3 in numpy; im ships pre-transposed in the
    exact SBUF layout.  The device runs ONLY the fp8 DoubleRow align
    matmuls, the DVE max-reduce, the tiny G accumulation and the hinge
    stats.  (v2 spent the first 16us of the kernel on device-side im
    normalization before the PE could start, plus per-tile gram matmuls
    + diag extraction + sqrt/reciprocal for the s norms.)
  * No s-scale anywhere: with s normalized on the host the G matrix
    entries are exactly 1/256 (power of two, exact in bf16), which
    cancels the 16*16 fp8 scaling, so s_acc accumulates scores at scale
    1 and the v2 hinge-stats epilogue is reused verbatim.
  * s rows are compacted globally (not per 128-sentence half): NT drops
    36 -> 35; the single half-boundary tile issues two G matmuls.
  * PSUM packing is flat: one [128, 512*NBANK] accumulation tile, im
    rows packed contiguously; matmuls split at bank boundaries (512
    f32), the DVE reduces view the flat range and may span banks, so a
    tile needs exactly one reduce instruction per R-class (4 of them).
  * mx is written by the DVE directly as bf16, feeding the G matmul
    with no scalar-engine hop.
  * DMA ramp: imt/gmat are split into 8-partition chunks issued from
    the scalar/gpsimd/vector queues in parallel with the sync queue's
    st stream (the first tiles split 4-way) so the first align matmul
    can start as soon as possible.
"""

import os
import sys

import numpy as np
import ml_dtypes

for _p in ("/opt/trn_rl_repo", "/root/.axon_site/_ro/trn_rl_repo"):
    if os.path.isdir(_p) and _p not in sys.path:
        sys.path.append(_p)

import concourse.bass as bass
import concourse.mybir as mybir
import concourse.tile as tile
from concourse import bacc
from concourse.bass_utils import run_bass_kernel_spmd


def _ensure_axon_hooks():
    """Some agent images ship an ``antenv`` without ``axon_hooks``, but
    bass_utils hard-imports it when trace=True.  Provide the registry and,
    when libaxon_pjrt.so is available, the real NTFF profile hook."""
    import types

    try:
        import antenv.axon_hooks  # noqa: F401
        return
    except ImportError:
        pass
    try:
        import antenv
    except ImportError:
        return
    mod = types.ModuleType("antenv.axon_hooks")
    mod._hook = None
    mod.set_axon_ntff_profile_hook = lambda h: setattr(mod, "_hook", h)
    mod.get_axon_ntff_profile_hook = lambda: mod._hook
    sys.modules["antenv.axon_hooks"] = mod
    antenv.axon_hooks = mod
    so_path = "/opt/axon/libaxon_pjrt.so"
    try:
        import trn_agent_boot.trn_boot as _tb
        if os.path.exists(so_path):
            mod._hook = _tb._ntff_profile_via_ctypes(so_path)
    except Exception:
        pass


_ensure_axon_hooks()

F32 = mybir.dt.float32
F32R = mybir.dt.float32r
BF16 = mybir.dt.bfloat16
F8 = mybir.dt.float8e4
I32 = mybir.dt.int32
AX = mybir.AxisListType
ALU = mybir.AluOpType
ACT = mybir.ActivationFunctionType
DR = mybir.MatmulPerfMode.DoubleRow

NCORES = 8
B, LI, LS, D = 256, 36, 30, 1024
KC = D // 128               # 8 contraction chunks of 128
G = 6                       # im row-padding granularity
MARGIN, EPS, NEG = 0.2, 1e-12, -1.0e9
GLAG = 8                    # tiles of lag before a tile's G matmul
SLAG = 3                    # extra lag for the stats PE-transpose part
GSC = 1.0 / 256.0           # exact in bf16; cancels the 16*16 fp8 scale
N_JUNK = int(os.environ.get("N_JUNK", "6"))  # PE warm-up matmuls

LAST_RESULT = None  # BassKernelResults of the most recent run (for test harness)

# Dedup redundant PE weight loads: bass lowering splits every matmul into a
# standalone Ldweights + non-self-loading Matmult, but emits one Ldweights
# per matmul even when consecutive matmuls share the same stationary
# operand.  We post-process the BIR json and drop a generated Ldweights
# (no semaphore waits/updates) when the weights signature matches what the
# PE already has loaded.
LDW_DEDUP = os.environ.get("LDW_DEDUP", "1") == "1"


def _dedup_ldweights_json(js_bytes):
    import json as _json

    j = _json.loads(js_bytes)
    dropped = 0
    for fn in j.get("functions", []):
        for blk in fn.get("blocks", []):
            insts = blk.get("instructions")
            if not insts:
                continue
            out = []
            loaded = None
            for x in insts:
                if x.get("engine") != "PE":
                    out.append(x)
                    continue
                op = x.get("opcode")
                if op == "Ldweights":
                    sig = _json.dumps(
                        [x.get("ins"), x.get("perf_mode"),
                         x.get("tile_size"), x.get("tile_position"),
                         x.get("is_transpose")], sort_keys=True)
                    sync = x.get("sync_info") or {}
                    if (sig == loaded and not sync.get("on_wait")
                            and not sync.get("on_update")):
                        dropped += 1
                        continue
                    loaded = sig
                    out.append(x)
                elif op == "Matmult":
                    if x.get("ldweights") is not False:
                        loaded = None  # self-loading matmul clobbers weights
                    out.append(x)
                else:
                    loaded = None
                    out.append(x)
            blk["instructions"] = out
    return _json.dumps(j).encode(), dropped


# ---------------------------------------------------------------------------
# layout planning (data-dependent, host side)
# ---------------------------------------------------------------------------

class Plan:
    pass


def plan_layout(im_l, s_l):
    p = Plan()
    # ---- s side: globally compacted row list ----
    rows = [(c, j) for c in range(B) for j in range(int(s_l[c]))]
    NT = -(-len(rows) // 128)
    rows = rows + [None] * (NT * 128 - len(rows))
    p.NT = NT
    p.srows = rows
    # per-tile sentence-half blocks for the G matmul.  Block t of gmat is
    # tile t's primary half; the (at most one) tile straddling the
    # c=127/128 boundary gets a second block at index NT.
    p.g_emits = []
    for t in range(NT):
        tr = [r for r in rows[128 * t:128 * t + 128] if r is not None]
        halves = sorted({r[0] // 128 for r in tr})
        ge = [(halves[0], t)]
        if len(halves) == 2:
            ge.append((halves[1], NT))
        p.g_emits.append(ge)
    p.first_t, p.last_t = {}, {}
    for t, ge in enumerate(p.g_emits):
        for h, _ in ge:
            p.first_t.setdefault(h, t)
            p.last_t[h] = t

    # ---- im side: R template shared across cores ----
    # R >= im_l+1 (>=1 zero row emulates the reference's max-includes-zero
    # mask) unless im_l == LI; multiple of G, clamped >= 18 so the template
    # has at most 4 R-classes -> 4 DVE reduce instructions per tile.
    R = np.where(im_l >= LI, LI,
                 (G * np.ceil((im_l + 1) / G)).astype(np.int64)).astype(np.int64)
    R = np.maximum(R, min(18, LI))
    order = np.argsort(-R, kind="stable")
    p.order = order                       # slot i of core m -> image order[8i+m]
    p.template = [int(R[order[8 * i]]) for i in range(32)]
    off = np.concatenate([[0], np.cumsum(p.template)]).astype(int)
    p.slot_off = off
    p.NR = int(off[32])
    p.NBANK = -(-p.NR // 512)
    assert p.NBANK * 512 <= 2048
    # reduce segments: runs of equal R (descending template -> contiguous)
    segs = []
    i = 0
    while i < 32:
        j = i
        while j < 32 and p.template[j] == p.template[i]:
            j += 1
        segs.append({"off": int(off[i]), "n": j - i, "R": p.template[i],
                     "mxoff": i})
        i = j
    p.segs = segs
    return p


def _plan_key(p):
    return (p.NT, p.NR, p.NBANK, tuple(p.template),
            tuple((t, h, blk) for t, ge in enumerate(p.g_emits)
                  for h, blk in ge))


# ---------------------------------------------------------------------------
# device program
# ---------------------------------------------------------------------------

def build_nc(p):
    NT, NR, NBANK = p.NT, p.NR, p.NBANK

    nc = bacc.Bacc(None, target_bir_lowering=False, debug=False,
                   num_devices=NCORES)

    imt_e = nc.declare_dram_parameter("imt", [128, KC * NR], F8,
                                      isOutput=False)
    st_e = nc.declare_dram_parameter("st", [NT, 128, KC, 128], F8,
                                     isOutput=False)
    gmat_e = nc.declare_dram_parameter("gmat", [128, (NT + 1) * 128], BF16,
                                       isOutput=False)
    ident_e = nc.declare_dram_parameter("ident", [128, 128], F32,
                                        isOutput=False)
    pos0_e = nc.declare_dram_parameter("pos0", [128, 32], F32, isOutput=False)
    pos1_e = nc.declare_dram_parameter("pos1", [128, 32], F32, isOutput=False)
    post0_e = nc.declare_dram_parameter("post0", [32, 128], F32, isOutput=False)
    post1_e = nc.declare_dram_parameter("post1", [32, 128], F32, isOutput=False)
    out_e = nc.declare_dram_parameter("out", [128, 6], F32, isOutput=True)

    with tile.TileContext(nc) as tc:
        from contextlib import ExitStack

        with ExitStack() as ctx:
            const = ctx.enter_context(tc.tile_pool(name="const", bufs=1))
            small = ctx.enter_context(tc.tile_pool(name="small", bufs=1))
            stp = ctx.enter_context(tc.tile_pool(name="stp", bufs=8))
            mxp = ctx.enter_context(tc.tile_pool(name="mxp", bufs=GLAG + 3))
            pal = ctx.enter_context(
                tc.tile_pool(name="pal", bufs=(3 if NBANK <= 2 else 2),
                             space="PSUM"))
            pmisc = ctx.enter_context(
                tc.tile_pool(name="pmisc", bufs=1, space="PSUM"))
            psacc = ctx.enter_context(
                tc.tile_pool(name="psacc", bufs=1, space="PSUM"))

            def misc_psum(shape, name):
                return pmisc.tile(shape, F32, tag="misc", bufs=1, name=name)

            # ---- PE warm-up: junk matmuls keep the PE p-state at max and
            # absorb the DMA ramp (weights memset by gpsimd at t~0) ----
            junkw = const.tile([128, 512], BF16, tag="junkw")
            nc.gpsimd.memset(junkw[:, :], 1.0)
            if N_JUNK:
                junk_ps = pmisc.tile([128, 512], F32, tag="misc", bufs=1,
                                     name="junk_ps")
                for _ in range(N_JUNK):
                    nc.tensor.matmul(junk_ps[:, :], lhsT=junkw[:, 0:128],
                                     rhs=junkw[:, :], start=True, stop=True,
                                     skip_group_check=True)

            # ---- ramp DMAs.  One dma_start per item (descriptors spread
            # round-robin over all 16 queues, so big DMAs transfer fast);
            # each sequencer blocks at ~4 outstanding DMAs, and readers wait
            # on per-queue completion counts, so items are issued strictly
            # in need-order and gmat/consts are deferred into the loop. ----
            imt_p = [const.tile([128, 2 * NR], F8, tag=f"imt{kp}",
                                name=f"imt{kp}")
                     for kp in range(KC // 2)]
            imt3_p = [x.rearrange("p (k n) -> p k n", k=2) for x in imt_p]
            gmat = const.tile([128, (NT + 1) * 128], BF16, tag="gmat")

            def issue_imt_piece(kp):
                e = nc.sync if kp < 2 else nc.gpsimd
                e.dma_start(out=imt_p[kp][:, :],
                            in_=imt_e[:, 2 * kp * NR:(2 * kp + 2) * NR])

            # ---- epilogue consts (gpsimd queue, issued mid-loop) ----
            ident = const.tile([128, 128], F32, tag="ident")
            pos0 = const.tile([128, 32], F32, tag="pos0")
            pos1 = const.tile([128, 32], F32, tag="pos1")
            post0 = const.tile([32, 128], F32, tag="post0")
            post1 = const.tile([32, 128], F32, tag="post1")
            margin128 = const.tile([128, 1], F32, tag="margin128")

            def issue_gmat():
                for ci in range(4):
                    nc.gpsimd.dma_start(
                        out=gmat[32 * ci:32 * ci + 32, :],
                        in_=gmat_e[32 * ci:32 * ci + 32, :])

            def issue_consts():
                nc.gpsimd.dma_start(out=ident[:, :], in_=ident_e[:, :])
                nc.gpsimd.dma_start(out=pos0[:, :], in_=pos0_e[:, :])
                nc.gpsimd.dma_start(out=pos1[:, :], in_=pos1_e[:, :])
                nc.gpsimd.dma_start(out=post0[:, :], in_=post0_e[:, :])
                nc.gpsimd.dma_start(out=post1[:, :], in_=post1_e[:, :])
                nc.gpsimd.memset(margin128[:, :], MARGIN)

            # gpsimd: the two late imt pieces (needed by tile 0's kp2/kp3)
            issue_imt_piece(2)
            issue_imt_piece(3)

            posm = [pos0, pos1]
            payload = small.tile([128, 6], F32, tag="payload")
            snd = [small.tile([128, 32], F32, tag=f"snd{h}", name=f"snd{h}")
                   for h in range(2)]
            trash32 = small.tile([128, 32], F32, tag="trash32")
            negm = [small.tile([128, 32], F32, tag=f"negm{h}", name=f"negm{h}")
                    for h in range(2)]
            stats_ready = []

            def ensure_stats_consts():
                # lazy: keeps the vector queue free during the DMA ramp
                if stats_ready:
                    return
                nc.vector.tensor_scalar_mul(negm[0][:, :], pos0[:, :], NEG)
                nc.vector.tensor_scalar_mul(negm[1][:, :], pos1[:, :], NEG)
                stats_ready.append(True)

            rm = small.tile([32, 2], F32, tag="rm")

            # S accumulators: both halves share one PSUM bank
            s_acc = psacc.tile([128, 64], F32, tag="S", name="S")
            s_ps = [s_acc[:, 0:32], s_acc[:, 32:64]]

            mx_tiles = {}

            def issue_st(t):
                st_t = stp.tile([128, KC * 128], F8, tag="st")
                st3 = st_t.rearrange("p (k c) -> p k c", k=KC)
                nc.sync.dma_start(out=st3[:, :, :], in_=st_e[t, :, :, :])
                return st_t

            def emit_tile(t, st_t):
                st3 = st_t.rearrange("p (k c) -> p k c", k=KC)
                # flat [128, NBANK*512] accumulation tile; matmuls split at
                # bank boundaries, reduces view the flat col range freely
                ps_t = pal.tile([128, NBANK * 512], F32, tag="al", name="ps")
                for kp in range(KC // 2):
                    w = st3[:, 2 * kp:2 * kp + 2, :]
                    for bi in range(NBANK):
                        c0, c1 = 512 * bi, min(512 * (bi + 1), NR)
                        nc.tensor.matmul(
                            ps_t[:, c0:c1],
                            lhsT=w,
                            rhs=imt3_p[kp][:, :, c0:c1],
                            start=(kp == 0), stop=(kp == KC // 2 - 1),
                            perf_mode=DR, skip_group_check=True,
                        )
                # max over image rows -> mx [128, 32] bf16 (feeds G matmul)
                mx = mxp.tile([128, 32], BF16, tag="mx", name="mx")
                for s in p.segs:
                    w = s["n"] * s["R"]
                    nc.vector.tensor_reduce(
                        out=mx[:, s["mxoff"]:s["mxoff"] + s["n"]],
                        in_=ps_t[:, s["off"]:s["off"] + w].rearrange(
                            "p (n r) -> p n r", r=s["R"]),
                        axis=AX.X, op=ALU.max,
                    )
                mx_tiles[t] = mx

            def emit_g(t):
                for h, blk in p.g_emits[t]:
                    nc.tensor.matmul(
                        s_ps[h],
                        lhsT=gmat[:, 128 * blk:128 * (blk + 1)],
                        rhs=mx_tiles[t][:, :],
                        start=(t == p.first_t[h]), stop=(t == p.last_t[h]),
                    )

            def emit_stats_a(h):
                # DVE part: diag extract, diag-masked copy, local col-max
                ensure_stats_consts()
                nc.vector.scalar_tensor_tensor(
                    out=trash32[:, :], in0=s_ps[h], scalar=1.0,
                    in1=posm[h][:, :], op0=ALU.mult, op1=ALU.mult,
                    accum_out=payload[:, 2 + h:3 + h],
                )
                nc.vector.tensor_add(snd[h][:, :], s_ps[h], negm[h][:, :])
                nc.vector.tensor_reduce(out=payload[:, h:h + 1],
                                        in_=snd[h][:, :], axis=AX.X,
                                        op=ALU.max)

            def emit_stats_b(h):
                # PE transpose trails the DVE part by SLAG tiles so the
                # in-order PE never waits on the DVE's reduce backlog
                stp_ps = misc_psum([32, 128], "stp_ps")
                nc.tensor.transpose(stp_ps[:, :], snd[h][:, :], ident[:, :])
                nc.vector.tensor_reduce(out=rm[:, h:h + 1], in_=stp_ps[:, :],
                                        axis=AX.X, op=ALU.max)

            # ---- main loop (G drained with GLAG-tile lag) ----
            done_a, done_b = set(), set()

            def after_g(t):
                for h in (0, 1):
                    if t == p.last_t[h] and h not in done_a:
                        emit_stats_a(h)
                        done_a.add(h)

            def after_b(t):
                for h in (0, 1):
                    if t == p.last_t[h] and h in done_a and h not in done_b:
                        emit_stats_b(h)
                        done_b.add(h)

            # sync queue order: st0, imt-kp0, st1, imt-kp1, st2, ... (need
            # order; all writers emitted before their readers below)
            st_pre = {0: issue_st(0)}
            issue_imt_piece(0)
            st_pre[1] = issue_st(1)
            issue_imt_piece(1)

            for t in range(NT):
                emit_tile(t, st_pre.pop(t) if t in st_pre else issue_st(t))
                if t == 2:
                    issue_gmat()
                if t == 4:
                    issue_consts()
                if t - GLAG >= 0:
                    emit_g(t - GLAG)
                    after_g(t - GLAG)
                if t - GLAG - SLAG >= 0:
                    after_b(t - GLAG - SLAG)
            for t in range(max(0, NT - GLAG), NT):
                emit_g(t)
                after_g(t)
            for h in (0, 1):
                if h not in done_b:
                    emit_stats_b(h)
                    done_b.add(h)

            # ---- row-hinge epilogue ----
            posr = [small.tile([128, 32], F32R, tag=f"posr{h}", name=f"posr{h}")
                    for h in range(2)]
            nc.scalar.copy(posr[0][:, :], pos0[:, :])
            nc.scalar.copy(posr[1][:, :], pos1[:, :])
            postr = [small.tile([32, 128], F32R, tag=f"postr{h}",
                                name=f"postr{h}") for h in range(2)]
            nc.scalar.copy(postr[0][:, :], post0[:, :])
            nc.scalar.copy(postr[1][:, :], post1[:, :])
            rowmax = small.tile([32, 1], F32, tag="rowmax")
            nc.vector.tensor_max(rowmax[:, :], rm[:, 0:1], rm[:, 1:2])
            # own-diag per image (row order): for each half h, pos_h^T @ d_h
            dca = small.tile([128, 2], F32R, tag="dca")
            dcb = small.tile([128, 2], F32R, tag="dcb")
            nc.scalar.copy(dca[:, 0:1], payload[:, 2:3])
            nc.scalar.mul(dca[:, 1:2], payload[:, 2:3], mul=0.0)
            nc.scalar.copy(dcb[:, 0:1], payload[:, 3:4])
            nc.scalar.mul(dcb[:, 1:2], payload[:, 3:4], mul=0.0)
            dfree_ps = misc_psum([32, 2], "dfree_ps")
            nc.tensor.matmul(dfree_ps[:, :], lhsT=posr[0][:, :],
                             rhs=dca[:, :], start=True, stop=False)
            nc.tensor.matmul(dfree_ps[:, :], lhsT=posr[1][:, :],
                             rhs=dcb[:, :], start=False, stop=True)
            dfree_sb = small.tile([32, 1], F32, tag="dfree_sb")
            nc.scalar.copy(dfree_sb[:, :], dfree_ps[:, 0:1])
            rh_pre = small.tile([32, 2], F32, tag="rh_pre")
            nc.gpsimd.memset(rh_pre[:, :], 0.0)
            nc.vector.tensor_sub(rh_pre[:, 0:1], rowmax[:, :], dfree_sb[:, :])
            rowhinge = small.tile([32, 2], F32R, tag="rowhinge")
            nc.scalar.activation(rowhinge[:, :], rh_pre[:, :], ACT.Relu,
                                 bias=margin128[0:32, :])
            for h in range(2):
                rh_ps = misc_psum([128, 2], "rh_ps")
                nc.tensor.matmul(rh_ps[:, :], lhsT=postr[h][:, :],
                                 rhs=rowhinge[:, :], start=True, stop=True)
                nc.scalar.copy(payload[:, 4 + h:5 + h], rh_ps[:, 0:1])

            nc.sync.dma_start(out=out_e[:, :], in_=payload[:, :])

    nc.finalize()
    return nc


# ---------------------------------------------------------------------------
# host side
# ---------------------------------------------------------------------------

def build_in_maps(p, im_set, s_seq):
    im_set = np.asarray(im_set, dtype=np.float32)
    s_seq = np.asarray(s_seq, dtype=np.float32)
    NT, NR = p.NT, p.NR

    # s tiles (shared): fp8 of 16*l2norm(word rows) in compacted order
    sn = s_seq / np.maximum(
        np.linalg.norm(s_seq, axis=2, keepdims=True), EPS)
    srows = np.zeros((NT * 128, D), dtype=np.float32)
    gmat = np.zeros((128, (NT + 1) * 128), dtype=np.float32)
    for i, cj in enumerate(p.srows):
        if cj is None:
            continue
        c, j = cj
        srows[i] = 16.0 * sn[c, 1 + j]
        t, pp = divmod(i, 128)
        h = c // 128
        blk = None
        for hh, bb in p.g_emits[t]:
            if hh == h:
                blk = bb
        gmat[pp, 128 * blk + (c % 128)] = GSC
    s8 = srows.astype(ml_dtypes.float8_e4m3)
    st = np.ascontiguousarray(
        s8.reshape(NT, 128, KC, 128).transpose(0, 3, 2, 1))
    gmat = gmat.astype(ml_dtypes.bfloat16)

    ident = np.eye(128, dtype=np.float32)

    imn = im_set / np.maximum(
        np.linalg.norm(im_set, axis=2, keepdims=True), EPS)

    in_maps = []
    for m in range(NCORES):
        imtf = np.zeros((NR, D), dtype=np.float32)
        pos0 = np.zeros((128, 32), np.float32)
        pos1 = np.zeros((128, 32), np.float32)
        for i in range(32):
            b = int(p.order[8 * i + m])
            off = int(p.slot_off[i])
            nvalid = int(p.im_l[b])
            imtf[off:off + nvalid] = 16.0 * imn[b, 1:1 + nvalid]
            if b < 128:
                pos0[b % 128, i] = 1.0
            else:
                pos1[b % 128, i] = 1.0
        imt8 = imtf.astype(ml_dtypes.float8_e4m3)
        imt = np.ascontiguousarray(
            imt8.reshape(NR, KC, 128).transpose(2, 1, 0)).reshape(128, KC * NR)
        in_maps.append({
            "imt": imt,
            "st": st,
            "gmat": gmat,
            "ident": ident,
            "pos0": pos0,
            "pos1": pos1,
            "post0": np.ascontiguousarray(pos0.T),
            "post1": np.ascontiguousarray(pos1.T),
        })
    return in_maps


def host_combine(outs):
    """Combine the 8 cores' [128, 6] payloads into the scalar loss."""
    agg = np.stack([np.asarray(o, dtype=np.float32) for o in outs])  # [8,128,6]
    colmax = agg[:, :, 0:2].max(axis=0)          # [128, 2]
    diag = agg[:, :, 2:4].sum(axis=0)            # [128, 2]
    colhinge = np.maximum(MARGIN + colmax - diag, 0.0).sum()
    rowhinge = agg[:, :, 4:6].sum()
    return np.float32(colhinge + rowhinge)


_NC_CACHE = {}


def kernel(im_set, s_seq, im_len, s_len):
    global LAST_RESULT
    im_len = np.asarray(im_len, dtype=np.int32)
    s_len = np.asarray(s_len, dtype=np.int32)
    im_l = im_len - 1
    s_l = s_len - 3

    p = plan_layout(im_l, s_l)
    p.im_l = im_l
    key = _plan_key(p)
    if key not in _NC_CACHE:
        nc = build_nc(p)
        if LDW_DEDUP:
            _orig = nc.to_json_bytes

            def _to_json_bytes_dedup(_orig=_orig):
                js, _ = _dedup_ldweights_json(_orig())
                return js

            nc.to_json_bytes = _to_json_bytes_dedup
        _NC_CACHE[key] = nc
    nc = _NC_CACHE[key]

    in_maps = build_in_maps(p, im_set, s_seq)
    res = run_bass_kernel_spmd(nc, in_maps, core_ids=list(range(NCORES)))
    LAST_RESULT = res
    return host_combine([r["out"] for r in res.results])
